# revision 45
# baseline (speedup 1.0000x reference)
"""Trainium2 Bass kernel for nn_CrossMultiheadAttention_44074954391814.

Sharding: 16 heads / 8 cores = 2 heads per core (128 of 1024 channels).
The batch-sum of attention is per-head, so with head sharding it stays
local to a core - no collective needed.  Each core reads the full x,y
(transposed + fp16 on host) and emits a partial (B*S, D) output (its
128-channel slice of the Wo contraction); the host sums the 8 partials
and adds the bias (bo + 4*bv@Wo.T - the v-bias contributes a constant
because each summed-attention row sums to exactly B=4).

Schedule (v2, h-outer two-stream):
 - head: x0 -> kproj(b0), y0 -> qproj(b0); warmup matmuls keep the HAM
   clock gate open through the DMA window.
 - S0 stream = all (b, h=0) softmax blocks, b-outer; micro-queue
   interleaves quarter DMAs, q/k projs (chasing DMA halves), v projs
   (resident quarter first), v transposes; b3 hosts lagged atrans(h0).
 - S1 stream = all (b, h=1) blocks, IT-outer so A[1] q-slices complete
   early: atrans(1,it) after each 4-block group; av chunks and the
   first outproj wave (st 0-3) run mid-stream so the 8.4MB output DMA
   is spread instead of draining at the end.
 - tail: atrans(1,7), av(h1, n=1), outproj st 4-7.
 - DVE normalize+accumulate fused via scalar_tensor_tensor
   (A = P*rinv + A in one pass); PSUM evacuations are spread across
   GpSimd (v/vtrans/atrans/outproj shares), DVE and ScalarE so the
   exp stream (64 x ~1.4us, the pacer) never waits.
"""

import sys

sys.path.insert(0, "/opt/trn_rl_repo")

from contextlib import ExitStack

import numpy as np

import concourse.bass as bass
import concourse.tile as tile
from concourse import bacc, mybir
from concourse.bass import ts
from concourse.bass_utils import run_bass_kernel_spmd
from concourse.masks import make_identity

D = 1024          # d_model
HEADS = 16
HD = 64           # head dim
B = 4
S = 1024
BS = B * S        # 4096
NCORES = 8
C = 128           # channels per core (2 heads * 64)
KT = D // 128     # 8 contraction tiles
FP16 = mybir.dt.float16
FP32 = mybir.dt.float32
SCALE = 1.0 / 8.0  # 1/sqrt(HD)
N_WARMUP = 26
MULT = mybir.AluOpType.mult
ADD = mybir.AluOpType.add


def build_program():
    nc = bacc.Bacc("TRN2", target_bir_lowering=False, debug=False)

    yT = nc.dram_tensor("yT", [D, BS], FP16, kind="ExternalInput").ap()
    xT = nc.dram_tensor("xT", [D, BS], FP16, kind="ExternalInput").ap()
    wqkvT = nc.dram_tensor("wqkvT", [D, 3 * C], FP16, kind="ExternalInput").ap()
    woT = nc.dram_tensor("woT", [C, D], FP16, kind="ExternalInput").ap()
    bqkv = nc.dram_tensor("bqkv", [C, 3], FP32, kind="ExternalInput").ap()
    out = nc.dram_tensor("out", [BS, D], FP16, kind="ExternalOutput").ap()

    with tile.TileContext(nc) as tc, ExitStack() as ctx:
        consts = ctx.enter_context(tc.tile_pool(name="consts", bufs=1))
        qk = ctx.enter_context(tc.tile_pool(name="qk", bufs=1))
        vpool = ctx.enter_context(tc.tile_pool(name="vpool", bufs=1))
        apool = ctx.enter_context(tc.tile_pool(name="apool", bufs=1))
        atpool = ctx.enter_context(tc.tile_pool(name="atpool", bufs=1))

        ident = consts.tile([128, 128], FP16, tag="ident")
        make_identity(nc, ident)

        wdummy = consts.tile([128, 512], FP16, tag="wdummy")
        nc.gpsimd.memset(wdummy, 0.0)

        wqkv_sb = consts.tile([128, KT, 3 * C], FP16, tag="wqkv")
        wo_sb = consts.tile([C, D], FP16, tag="wo")
        bqkv_sb = consts.tile([C, 3], FP32, tag="bqkv")

        def load_w(wi):
            # per-matrix weight loads so only Wk (0.25MB) sits ahead of
            # x0 in the single DMA queue on the first-exp critical path
            nc.sync.dma_start(
                out=wqkv_sb[:, :, wi * C : (wi + 1) * C],
                in_=wqkvT[:, wi * C : (wi + 1) * C].rearrange(
                    "(kt p) c -> p kt c", p=128
                ),
            )

        qT = qk.tile([C, BS], FP16, tag="qT")
        kT = qk.tile([C, BS], FP16, tag="kT")
        vT = qk.tile([C, BS], FP16, tag="vT")
        vpair = vpool.tile([128, 2, 2, 8, 128], FP16, tag="vpair")

        A = apool.tile([128, 2, S // 128, S], FP16, tag="A")
        AT = atpool.tile([128, 2, S // 128, S], FP16, tag="AT")

        with (
            tc.tile_pool(name="xy", bufs=4) as xy,
            tc.tile_pool(name="pp_qkv", bufs=3, space="PSUM") as pp_qkv,
            tc.tile_pool(name="tp", bufs=1, space="PSUM") as tp,
            tc.tile_pool(name="pp_sc", bufs=2, space="PSUM") as pp_sc,
            tc.tile_pool(name="ppool", bufs=6) as ppool,
            tc.tile_pool(name="rpool", bufs=12) as rpool,
            tc.tile_pool(name="ovpool", bufs=4) as ovpool,
            tc.tile_pool(name="opool", bufs=3) as opool,
        ):
            def load_quarter(src_dram, g, tag):
                q = xy.tile([128, KT, 1024], FP16, tag=tag, name=f"xy_{tag}_{g}")
                nc.sync.dma_start(
                    out=q,
                    in_=src_dram[:, g * 1024 : (g + 1) * 1024].rearrange(
                        "(kt p) s -> p kt s", p=128
                    ),
                )
                return q

            def load_cols(q, src_dram, g, c0, w):
                nc.sync.dma_start(
                    out=q[:, :, c0 : c0 + w],
                    in_=src_dram[
                        :, g * 1024 + c0 : g * 1024 + c0 + w
                    ].rearrange("(kt p) s -> p kt s", p=128),
                )

            def warm(n=1, w=256):
                # dummy matmuls with no cross-engine deps: keep the PE
                # continuously busy (the clock ramps 0.65->1.2->2.4 GHz
                # and needs 3us of uninterrupted execution for full
                # speed; any idle gap resets it to 1.2 GHz).  Uses the
                # score-psum ring, so only call while scores are idle
                # (head / tail).
                for _ in range(n):
                    wp = pp_sc.tile([128, S], FP32, tag="sc", name="warm")
                    nc.tensor.matmul(
                        wp[:, 0:w], lhsT=wdummy[:, 0:128], rhs=wdummy[:, 0:w],
                        start=True, stop=True,
                    )

            def proj_part(state, src_q, wi, dst, g, c0, w, part):
                # half-group emission (4 matmuls) so softmax score pairs
                # never sit behind a full 8-matmul group in the PE FIFO
                if part == 0:
                    state["ps"] = pp_qkv.tile(
                        [C, 512], FP32, tag="ps", name="ps"
                    )
                ps = state["ps"][:, 0:w]
                for kt in range(4 * part, 4 * part + 4):
                    nc.tensor.matmul(
                        ps,
                        lhsT=wqkv_sb[:, kt, wi * C : (wi + 1) * C],
                        rhs=src_q[:, kt, c0 : c0 + w],
                        start=(kt == 0),
                        stop=(kt == KT - 1),
                    )
                if part == 1:
                    dstap = dst[:, g * 1024 + c0 : g * 1024 + c0 + w]
                    if wi == 2:
                        # v-bias folded into the host-side output bias;
                        # ScalarE stays exp-only during S0
                        nc.vector.tensor_copy(dstap, ps)
                    else:
                        nc.vector.tensor_scalar_add(
                            out=dstap, in0=ps, scalar1=bqkv_sb[:, wi : wi + 1]
                        )

            def proj_group(src_q, wi, dst, g, c0, w=512):
                st = {}
                proj_part(st, src_q, wi, dst, g, c0, w, 0)
                proj_part(st, src_q, wi, dst, g, c0, w, 1)

            def proj_halves(src_q, wi, dst, g, n2):
                st = {}
                return [
                    lambda p=p: proj_part(st, src_q, wi, dst, g, n2 * 512, 512, p)
                    for p in range(2)
                ]

            def vtrans_part(state, g, part):
                if part == 0:
                    state["vps"] = tp.tile(
                        [128, 1024], FP16, tag="tp", name="vps"
                    )
                vps = state["vps"]
                for k in range(4 * part, 4 * part + 4):
                    nc.tensor.matmul(
                        vps[:, ts(k, 128)],
                        lhsT=vT[:, ts(g * 8 + k, 128)],
                        rhs=ident,
                        is_transpose=True,
                        start=(k == 0),
                        stop=(k == 7),
                    )
                if part == 1:
                    vps3 = vps.rearrange("p (jt c) -> p jt c", jt=8)
                    for h in range(2):
                        dst = vpair[
                            :, h, g // 2, :,
                            (g % 2) * 64 : (g % 2) * 64 + 64,
                        ]
                        src = vps3[:, :, h * 64 : h * 64 + 64]
                        nc.vector.tensor_copy(dst, src)

            def vtrans_halves(g):
                st = {}
                return [lambda p=p: vtrans_part(st, g, p) for p in range(2)]

            def softmax_block(b, h, it):
                sc = pp_sc.tile([128, S], FP32, tag="sc", name="sc")
                for jt in range(2):
                    nc.tensor.matmul(
                        sc[:, ts(jt, 512)],
                        lhsT=qT[
                            h * 64 : h * 64 + 64,
                            b * S + it * 128 : b * S + (it + 1) * 128,
                        ],
                        rhs=kT[
                            h * 64 : h * 64 + 64,
                            b * S + jt * 512 : b * S + (jt + 1) * 512,
                        ],
                        start=True,
                        stop=True,
                    )
                P = ppool.tile([128, S], FP16, tag="P")
                r = rpool.tile([128, 1], FP32, tag="r")
                rinv = rpool.tile([128, 1], FP32, tag="rinv")
                nc.scalar.activation(
                    out=P,
                    in_=sc,
                    func=mybir.ActivationFunctionType.Exp,
                    scale=SCALE,
                    accum_out=r,
                )
                nc.vector.reciprocal(out=rinv, in_=r)
                if b == 0:
                    nc.vector.tensor_scalar_mul(
                        out=A[:, h, it, :], in0=P, scalar1=rinv
                    )
                elif b == 1 and it % 2 == 0:
                    # gpsimd takes half the b==1 accumulates (SBUF-only
                    # engine, no PSUM access, no scalar_tensor_tensor
                    # opcode - so DVE scales, gpsimd adds)
                    Pw = ppool.tile([128, S], FP16, tag="Pw", bufs=4)
                    nc.vector.tensor_scalar_mul(out=Pw, in0=P, scalar1=rinv)
                    nc.gpsimd.tensor_add(A[:, h, it, :], A[:, h, it, :], Pw)
                else:
                    # fused normalize+accumulate on DVE: A = P*rinv + A
                    nc.vector.scalar_tensor_tensor(
                        out=A[:, h, it, :],
                        in0=P,
                        scalar=rinv,
                        in1=A[:, h, it, :],
                        op0=MULT,
                        op1=ADD,
                    )

            def atrans_block(h, it, evac):
                aps = tp.tile([128, 1024], FP16, tag="tp", name="aps")
                for jt in range(8):
                    nc.tensor.matmul(
                        aps[:, ts(jt, 128)],
                        lhsT=A[:, h, it, ts(jt, 128)],
                        rhs=ident,
                        is_transpose=True,
                        start=(jt == 0),
                        stop=(jt == 7),
                    )
                aps3 = aps.rearrange("c (jt p) -> c jt p", jt=8)
                if evac == "v":
                    nc.vector.tensor_copy(AT[:, h, :, ts(it, 128)], aps3)
                else:
                    nc.scalar.copy(AT[:, h, :, ts(it, 128)], aps3)

            ovT = [
                ovpool.tile([C, S], FP16, tag="ovT", name=f"ovT{b}")
                for b in range(B)
            ]

            def av_chunk(h, p, q0, qw=512, e0="s", e1="v"):
                av = pp_qkv.tile([128, 512], FP32, tag="ps", name="av")
                for jt in range(8):
                    nc.tensor.matmul(
                        av[:, 0:qw],
                        lhsT=vpair[:, h, p, jt, :],
                        rhs=AT[:, h, jt, q0 : q0 + qw],
                        start=(jt == 0),
                        stop=(jt == 7),
                    )
                for half, eng in ((0, e0), (1, e1)):
                    b = 2 * p + half
                    dst = ovT[b][h * 64 : h * 64 + 64, q0 : q0 + qw]
                    src = av[half * 64 : half * 64 + 64, 0:qw]
                    if eng == "s":
                        nc.scalar.copy(dst, src)
                    else:
                        nc.vector.tensor_copy(dst, src)

            def outproj_chunk(b, st, engs=("v", "g")):
                o_sb = opool.tile([128, D], FP16, tag="osb")
                for n in range(2):
                    o_ph = pp_qkv.tile([128, 512], FP32, tag="ps", name="oph")
                    nc.tensor.matmul(
                        o_ph,
                        lhsT=ovT[b][:, ts(st, 128)],
                        rhs=wo_sb[:, ts(n, 512)],
                        start=True,
                        stop=True,
                    )
                    eng = engs[n]
                    dst = o_sb[:, ts(n, 512)]
                    if eng == "v":
                        nc.vector.tensor_copy(dst, o_ph)
                    else:
                        nc.scalar.copy(dst, o_ph)
                nc.sync.dma_start(
                    out=out[b * S + st * 128 : b * S + (st + 1) * 128, :],
                    in_=o_sb,
                )

            # ---------------- head ----------------
            yq = xy.tile([128, KT, 1024], FP16, tag="xyq", name="xy_y0")
            xq = xy.tile([128, KT, 1024], FP16, tag="xyq", name="xy_x0")

            nc.sync.dma_start(out=bqkv_sb, in_=bqkv)
            load_w(1)
            load_cols(xq, xT, 0, 0, 512)
            load_cols(xq, xT, 0, 512, 512)
            load_w(0)
            load_cols(yq, yT, 0, 0, 256)
            load_w(2)
            load_cols(yq, yT, 0, 256, 256)
            load_cols(yq, yT, 0, 512, 512)
            nc.sync.dma_start(out=wo_sb, in_=woT)
            # chase the DMA arrivals (single queue, in issue order);
            # warm matmuls bridge every wait so the PE clock ramp
            # never resets before the exp stream starts
            warm(N_WARMUP)
            proj_group(xq, 1, kT, 0, 0)
            warm(9)
            proj_group(xq, 1, kT, 0, 512)
            warm(2)
            proj_group(yq, 0, qT, 0, 0, 256)
            warm(1)
            proj_group(yq, 0, qT, 0, 256, 256)

            # ------- S0: batches 0,1,2 - (h0,h1) pairs, b-outer -------
            # same-batch head pairing keeps both PE row groups (64-row
            # score matmuls) active so the HAM clock stays at full speed
            xqs = {0: xq}
            yqs = {0: yq}
            queue = []
            done = {"n": 0}

            def pops(k):
                while k > 0 and done["n"] < len(queue):
                    queue[done["n"]]()
                    done["n"] += 1
                    k -= 1

            def ldx(g):
                xqs[g] = load_quarter(xT, g, "xyq")

            def ldy(g):
                yqs[g] = load_quarter(yT, g, "xyq")

            def vproj_items(g):
                # xqs[g] resolved lazily at call time
                items = []
                for n2 in range(2):
                    st2 = {}
                    for p in range(2):
                        items.append(
                            lambda st2=st2, g=g, n2=n2, p=p: proj_part(
                                st2, xqs[g], 2, vT, g, n2 * 512, 512, p
                            )
                        )
                return items

            def qkproj_items(g, combos=None):
                items = []
                for wi, dst, n2 in combos or (
                    (1, kT, 0), (0, qT, 0), (1, kT, 1), (0, qT, 1)
                ):
                    st2 = {}
                    for p in range(2):
                        items.append(
                            lambda st2=st2, wi=wi, dst=dst, n2=n2, p=p,
                            g=g: proj_part(
                                st2, (xqs if wi else yqs)[g], wi, dst,
                                g, n2 * 512, 512, p
                            )
                        )
                return items

            for g in range(1, B):
                queue.append(lambda g=g: ldx(g))
                queue.append(lambda g=g: ldy(g))
                # previous batch's v-proj first: resident data, never
                # head-of-line blocks the PE on the fresh quarter DMA
                queue.extend(vproj_items(g - 1))
                if g == 1:
                    # q(b0) cols 512.. lands late in the single DMA
                    # queue; not needed before block (b0, h, it=4)
                    queue.extend(proj_halves(yqs[0], 0, qT, 0, 1))
                queue.extend(vtrans_halves(g - 1))
                if g == B - 1:
                    # q(b3) n1 deferred to S1 (needed from group 4)
                    queue.extend(
                        qkproj_items(g, ((1, kT, 0), (0, qT, 0), (1, kT, 1)))
                    )
                else:
                    queue.extend(qkproj_items(g))

            # S0 = b0 (h0,h1) pairs + b1-h0 + b2-h0: 32 blocks that
            # chase the input DMA while hosting all projections.  The
            # h1 halves of b1/b2 are DEFERRED to S1 so that every
            # (h, it) A-slice finishes inside S1's stream and the
            # av/outproj waves (and their 8.4MB of output DMA) overlap
            # the exp stream instead of draining at the end.
            for it in range(S // 128):
                pops(2)
                softmax_block(0, 0, it)
                pops(1)
                softmax_block(0, 1, it)
            for bb in (1, 2):
                for it in range(S // 128):
                    pops(2)
                    softmax_block(bb, 0, it)

            # b3's v-proj/transpose and q-n1 seed S1's first groups so
            # the PE has pop work there (av waves only start at group 3)
            queue.extend(qkproj_items(3, ((0, qT, 1),)))
            queue.extend(vproj_items(3))
            queue.extend(vtrans_halves(3))

            def warmq(n=1, w=512):
                # in-stream warm filler on the pp_qkv ring (the score
                # ring is live during S1, the proj ring mostly idle)
                for _ in range(n):
                    wp = pp_qkv.tile([128, 512], FP32, tag="ps", name="wq")
                    nc.tensor.matmul(
                        wp[:, 0:w], lhsT=wdummy[:, 0:128],
                        rhs=wdummy[:, 0:w], start=True, stop=True,
                    )

            # ------- S1: finisher stream -------
            # group it: (b1,h1) (b3,h0) (b2,h1) (b3,h1) - after the
            # group, A[0,it] and A[1,it] are complete; atrans lags one
            # it; av waves are 256 q-cols so outproj waves start early
            def wave_items(q4):
                items = []
                for p in range(2):
                    items.append(
                        lambda p=p: av_chunk(0, p, q4 * 256, 256, "v", "v")
                    )
                    items.append(
                        lambda p=p: av_chunk(1, p, q4 * 256, 256, "v", "v")
                    )
                    for bb in (2 * p, 2 * p + 1):
                        for st in (2 * q4, 2 * q4 + 1):
                            items.append(
                                lambda bb=bb, st=st: outproj_chunk(
                                    bb, st,
                                    ("s", "v") if (bb + st) % 2
                                    else ("v", "v"),
                                )
                            )
                return items

            for it in range(S // 128):
                np_ = 3 if it == 7 else 2
                if it >= 1:
                    atrans_block(0, it - 1, "v" if it % 2 else "s")
                pops(np_)
                softmax_block(1, 1, it)
                pops(np_)
                softmax_block(3, 0, it)
                if it >= 1:
                    atrans_block(1, it - 1, "v" if it % 2 else "s")
                pops(np_)
                softmax_block(2, 1, it)
                pops(np_)
                softmax_block(3, 1, it)
                if it <= 2:
                    warmq(2)
                if it in (2, 4, 6):
                    # wave q4 only after BOTH lag-1 atrans of q-slice
                    # 2*q4+1 were emitted (group 2*q4+2) - popping it
                    # earlier would read not-yet-written AT slices
                    queue.extend(wave_items((it - 2) // 2))

            # ---------------- tail ----------------
            # critical chain: last STT -> atrans(*,7) -> av q4=3 ->
            # outproj st6/7; warms bridge every cross-engine wait so
            # the PE clock never drops during the drain
            warm(2)
            atrans_block(0, 7, "s")
            warm(1)
            atrans_block(1, 7, "v")
            pops(len(queue))
            warm(2)
            av_chunk(0, 0, 768, 256, "s", "v")
            warm(1)
            av_chunk(1, 0, 768, 256, "v", "s")
            warm(1)
            av_chunk(0, 1, 768, 256, "s", "v")
            outproj_chunk(0, 6, ("s", "v"))
            av_chunk(1, 1, 768, 256, "v", "s")
            outproj_chunk(1, 6, ("v", "s"))
            outproj_chunk(0, 7, ("s", "v"))
            outproj_chunk(1, 7, ("v", "s"))
            warm(1)
            outproj_chunk(2, 6, ("s", "v"))
            outproj_chunk(3, 6, ("v", "s"))
            warm(1)
            outproj_chunk(2, 7, ("s", "v"))
            outproj_chunk(3, 7, ("v", "s"))

    return nc


_PROGRAM = None


def _get_program():
    global _PROGRAM
    if _PROGRAM is None:
        _PROGRAM = build_program()
        _PROGRAM.finalize()
    return _PROGRAM


def _host_in_maps(x, y, Wq, Wk, Wv, Wo, bq, bk, bv):
    xT16 = np.ascontiguousarray(x.reshape(BS, D).T).astype(np.float16)
    yT16 = np.ascontiguousarray(y.reshape(BS, D).T).astype(np.float16)
    in_maps = []
    for c in range(NCORES):
        rows = slice(c * C, (c + 1) * C)
        wqkv = np.concatenate(
            [Wq[rows, :].T, Wk[rows, :].T, Wv[rows, :].T], axis=1
        )
        bqkv = np.stack([bq[rows], bk[rows], bv[rows]], axis=1)
        in_maps.append(
            {
                "yT": yT16,
                "xT": xT16,
                "wqkvT": np.ascontiguousarray(wqkv).astype(np.float16),
                "woT": np.ascontiguousarray(Wo[:, rows].T).astype(np.float16),
                "bqkv": np.ascontiguousarray(bqkv).astype(np.float32),
            }
        )
    return in_maps


def kernel(**inputs):
    x = np.asarray(inputs["x"], dtype=np.float32)
    y = np.asarray(inputs["y"], dtype=np.float32)
    Wq = np.asarray(inputs["Wq"], dtype=np.float32)
    Wk = np.asarray(inputs["Wk"], dtype=np.float32)
    Wv = np.asarray(inputs["Wv"], dtype=np.float32)
    Wo = np.asarray(inputs["Wo"], dtype=np.float32)
    bq = np.asarray(inputs["bq"], dtype=np.float32)
    bk = np.asarray(inputs["bk"], dtype=np.float32)
    bv = np.asarray(inputs["bv"], dtype=np.float32)
    bo = np.asarray(inputs["bo"], dtype=np.float32)

    in_maps = _host_in_maps(x, y, Wq, Wk, Wv, Wo, bq, bk, bv)
    nc = _get_program()
    res = run_bass_kernel_spmd(nc, in_maps, list(range(NCORES)))

    acc = np.zeros((BS, D), dtype=np.float32)
    for c in range(NCORES):
        acc += res.results[c]["out"].astype(np.float32)
    # v-bias folded here: rows of the batch-summed attention sum to B
    acc += (bo + float(B) * (bv @ Wo.T))[None, :]
    return acc.reshape(B, S, D)
